# revision 5
# baseline (speedup 1.0000x reference)
"""Trainium2 Bass kernel for a Hyena-style decode block (HCLDecodeBlock).

Tensor-parallel over 8 NeuronCores:
  - proj_w / mlp_w1 / mlp_w3 column-sharded, out_w / mlp_w2 row-sharded
  - FIR/IIR states sharded along the channel axis with the head split
  - one on-device AllReduce after out_w; the mlp_w2 row-parallel partial
    sums are reduced on the host during the output gather.

Big matmul weights are cast to bf16 on the host to halve HBM traffic
(the problem is memory-bound); all states/outputs stay f32.
"""

import sys

for _p in ("/opt/trn_rl_repo",):
    if _p not in sys.path:
        sys.path.insert(0, _p)

import numpy as np
import ml_dtypes

import concourse.bass as bass
import concourse.mybir as mybir
import concourse.tile as tile
from concourse import bacc
from concourse.bass_utils import run_bass_kernel_spmd

F32 = mybir.dt.float32
BF16 = mybir.dt.bfloat16
AX = mybir.AxisListType
OP = mybir.AluOpType
ACT_F = mybir.ActivationFunctionType

B, D, S, FL, NH, FF = 8, 4096, 16, 3, 32, 16384
NCORES = 8
CP = 3 * D // NCORES    # 1536 proj cols per core
DP = D // NCORES        # 512 channels per core
FFP = FF // NCORES      # 2048 ff cols per core
HP = NH // NCORES       # 4 heads per core
EPS = 1e-6
BF = np.dtype(ml_dtypes.bfloat16)


def _bcast_last(ap, n):
    """Append a broadcast (step 0) innermost dim of size n to an AP."""
    return bass.AP(ap.tensor, ap.offset, list(ap.ap) + [[0, n]])


def build_program():
    nc = bacc.Bacc("TRN2", target_bir_lowering=False, debug=False,
                   num_devices=NCORES)

    def inp(name, shape, dt=F32):
        return nc.dram_tensor(name, list(shape), dt, kind="ExternalInput")

    def outp(name, shape, dt=F32):
        return nc.dram_tensor(name, list(shape), dt, kind="ExternalOutput")

    x_g     = inp("x_g", (B, D))
    xpb_g   = inp("xpb_g", (B, D))              # x + out_b (residual base)
    projw   = inp("projw", (D, CP), BF16)
    taps    = inp("taps", (B, 4 * CP))          # [tap0 | tap1 | tap2(h0) | bias]
    fir_in  = inp("fir_in", (B, 2 * CP))        # (c, j) interleaved
    polesT  = inp("polesT", (DP, B * S))        # [c, b*S+s]
    residT  = inp("residT", (DP, B * S))
    dresT   = inp("dresT", (DP, 1))
    iir_in  = inp("iir_in", (B, DP, S))
    outw    = inp("outw", (DP, D), BF16)
    pnw_t   = inp("pnw_t", (128, D // 128))
    psw_t   = inp("psw_t", (128, D // 128))
    w1      = inp("w1", (D, FFP), BF16)
    w3      = inp("w3", (D, FFP), BF16)
    w2      = inp("w2", (FFP, D), BF16)

    u_out    = outp("u_out", (B, CP))
    iir_out  = outp("iir_out", (B, DP, S))
    xmid_out = outp("xmid_out", (B, D))
    mlp_out  = outp("mlp_out", (B, D))

    KT = D // 128      # 32 k-tiles over D
    KT2 = FFP // 128   # 16 k-tiles over ff shard

    with tile.TileContext(nc) as tc:
        with (
            tc.tile_pool(name="const", bufs=1) as const,
            tc.tile_pool(name="bv", bufs=1) as bv,
            tc.tile_pool(name="fir", bufs=1) as firp,
            tc.tile_pool(name="cl", bufs=1) as cl,
            tc.tile_pool(name="iirp", bufs=2) as iirp,
            tc.tile_pool(name="act", bufs=1) as actp,
            tc.tile_pool(name="small", bufs=1) as small,
            tc.tile_pool(name="pw", bufs=3) as pw_pool,
            tc.tile_pool(name="ow", bufs=2) as ow_pool,
            tc.tile_pool(name="w1p", bufs=5) as w1_pool,
            tc.tile_pool(name="w3p", bufs=5) as w3_pool,
            tc.tile_pool(name="w2p", bufs=3) as w2_pool,
            tc.tile_pool(name="pt", bufs=4, space="PSUM") as pt_pool,
            tc.tile_pool(name="pmm", bufs=2, space="PSUM") as pmm_pool,
            tc.tile_pool(name="dram", bufs=1, space="DRAM") as dram,
        ):
            # ---- constants ----
            ident = const.tile([128, 128], F32)
            from concourse.masks import make_identity
            make_identity(nc, ident[:, :])
            pnw_sb = const.tile([128, D // 128], F32)
            nc.scalar.dma_start(pnw_sb[:], pnw_t.ap())
            psw_sb = const.tile([128, D // 128], F32)
            nc.scalar.dma_start(psw_sb[:], psw_t.ap())
            dres_sb = const.tile([128, DP // 128], F32)
            nc.scalar.dma_start(
                dres_sb[:], dresT.ap().rearrange("(t p) o -> p (t o)", p=128))
            eps_sb = const.tile([B, 1], F32)
            nc.vector.memset(eps_sb[:], EPS)

            # Three rotating 16KB-per-partition scratch slots (tags sA/sB/sC):
            #   sA: xt -> m(FIR scratch) -> xpb -> xs2
            #   sB: sq -> pa_sb -> sq2
            #   sC: xs -> x_mid -> mlp_sb
            # ---- phase 0: load x, rmsnorm scale ----
            xt = bv.tile([B, D], F32, tag="sA", name="xt")
            nc.scalar.dma_start(xt[:], x_g.ap())
            sq = bv.tile([B, D], F32, tag="sB", name="sq")
            ssq = small.tile([B, 1], F32, tag="ssq")
            nc.scalar.activation(sq[:], xt[:], ACT_F.Square, accum_out=ssq[:])
            rstd = small.tile([B, 1], F32, tag="rstd")
            nc.scalar.activation(rstd[:], ssq[:], ACT_F.Sqrt,
                                 bias=eps_sb[:], scale=1.0 / D)
            nc.vector.reciprocal(rstd[:], rstd[:])
            xs = bv.tile([B, D], F32, tag="sC", name="xs")
            nc.vector.tensor_scalar_mul(xs[:], xt[:], rstd[:])

            # ---- phase 1: transpose xs -> xnT (bf16), fused pre_norm_w ----
            xnT = actp.tile([128, KT, B], BF16, tag="xnT")
            for kt in range(KT):
                tp = pt_pool.tile([128, B], F32, tag="tp", name="tp")
                nc.tensor.transpose(tp[:], xs[:, kt * 128:(kt + 1) * 128],
                                    ident[:B, :B])
                nc.scalar.mul(xnT[:, kt, :], tp[:], pnw_sb[:, kt:kt + 1])

            # ---- phase 2: proj matmul -> u (B, CP) ----
            u_ps = pmm_pool.tile([128, 512], F32, tag="mm", name="u_ps")
            for kt in range(KT):
                wt = pw_pool.tile([128, CP], BF16, tag="pw", name="wt")
                nc.sync.dma_start(wt[:], projw.ap()[kt * 128:(kt + 1) * 128, :])
                for n in range(CP // 512):
                    nc.tensor.matmul(
                        u_ps[32 * n:32 * n + B, :],
                        xnT[:, kt, :], wt[:, n * 512:(n + 1) * 512],
                        start=(kt == 0), stop=(kt == KT - 1),
                        tile_position=(0, 32 * n))
            u_sb = firp.tile([B, CP], F32, tag="u")
            for n in range(CP // 512):
                nc.scalar.copy(u_sb[:, n * 512:(n + 1) * 512],
                               u_ps[32 * n:32 * n + B, :])
            nc.scalar.dma_start(u_out.ap(), u_sb[:])

            # ---- phase 3: FIR ----
            taps_sb = firp.tile([B, 4 * CP], F32, tag="taps")
            nc.scalar.dma_start(taps_sb[:], taps.ap())
            fir_sb = firp.tile([B, 2 * CP], F32, tag="firg", name="fir_sb")
            nc.scalar.dma_start(fir_sb[:], fir_in.ap())
            z_sb = firp.tile([B, CP], F32, tag="zs", name="z_sb")
            m_bv = bv.tile([B, D], F32, tag="sA", name="m_bv")
            m_sb = m_bv[:, :CP]
            fir3 = fir_sb[:].rearrange("b (c j) -> b c j", j=2)
            nc.vector.tensor_mul(z_sb[:], u_sb[:], taps_sb[:, 2 * CP:3 * CP])
            nc.vector.tensor_mul(m_sb, fir3[:, :, 0], taps_sb[:, 0:CP])
            nc.vector.tensor_add(z_sb[:], z_sb[:], m_sb)
            nc.vector.tensor_mul(m_sb, fir3[:, :, 1], taps_sb[:, CP:2 * CP])
            nc.vector.tensor_add(z_sb[:], z_sb[:], m_sb)
            nc.vector.tensor_add(z_sb[:], z_sb[:], taps_sb[:, 3 * CP:4 * CP])

            # ---- phase 4: head split + transposes to channel layout ----
            x2T = cl.tile([128, HP, B], F32, tag="x2T")
            x1T = cl.tile([128, HP, B], F32, tag="x1T")
            vT = cl.tile([128, HP, B], F32, tag="vT")
            for h in range(HP):
                base = h * (3 * 128)
                for dst, off in ((x2T, 0), (x1T, 128), (vT, 256)):
                    tp = pt_pool.tile([128, B], F32, tag="tp", name="tp")
                    nc.tensor.transpose(
                        tp[:], z_sb[:, base + off:base + off + 128],
                        ident[:B, :B])
                    nc.scalar.copy(dst[:, h, :], tp[:])
            x1vT = cl.tile([128, HP, B], F32, tag="x1vT")
            nc.vector.tensor_mul(x1vT[:], x1T[:], vT[:])

            # ---- phase 5: IIR (channel-partition layout) ----
            resT = cl.tile([128, HP, B], F32, tag="resT")
            iir_r = iir_in.ap().rearrange("b c s -> c b s")
            iiro_r = iir_out.ap().rearrange("b c s -> c b s")
            for ct in range(DP // 128):
                it = iirp.tile([128, B, S], F32, tag="it", name="it")
                nc.scalar.dma_start(it[:], iir_r[ct * 128:(ct + 1) * 128])
                po = iirp.tile([128, B * S], F32, tag="po", name="po")
                nc.scalar.dma_start(po[:], polesT.ap()[ct * 128:(ct + 1) * 128, :])
                nc.vector.tensor_mul(it[:], it[:],
                                     po[:].rearrange("p (b s) -> p b s", s=S))
                nc.vector.tensor_tensor(
                    it[:], it[:], _bcast_last(x1vT[:, ct, :], S), OP.add)
                nc.scalar.dma_start(iiro_r[ct * 128:(ct + 1) * 128], it[:])
                rr = iirp.tile([128, B * S], F32, tag="rr", name="rr")
                nc.scalar.dma_start(rr[:], residT.ap()[ct * 128:(ct + 1) * 128, :])
                nc.vector.tensor_mul(it[:], it[:],
                                     rr[:].rearrange("p (b s) -> p b s", s=S))
                nc.vector.tensor_reduce(resT[:, ct, :], it[:], AX.X, OP.add)

            # ---- phase 6: y = x2 * (res + D_res * x1v), bf16, c-layout ----
            yT = cl.tile([128, HP, B], BF16, tag="yT")
            for ct in range(DP // 128):
                nc.vector.scalar_tensor_tensor(
                    resT[:, ct, :], x1vT[:, ct, :], dres_sb[:, ct:ct + 1],
                    resT[:, ct, :], OP.mult, OP.add)
            nc.vector.tensor_mul(yT[:], x2T[:], resT[:])

            # ---- phase 7: out proj partial + AllReduce ----
            o_ps = [pmm_pool.tile([128, 512], F32, tag="mm", name=f"o_ps{i}")
                    for i in range(2)]
            for ct in range(DP // 128):
                owt = ow_pool.tile([128, D], BF16, tag="ow", name="owt")
                nc.sync.dma_start(owt[:], outw.ap()[ct * 128:(ct + 1) * 128, :])
                for n in range(D // 512):
                    nc.tensor.matmul(
                        o_ps[n // 4][32 * (n % 4):32 * (n % 4) + B, :],
                        yT[:, ct, :], owt[:, n * 512:(n + 1) * 512],
                        start=(ct == 0), stop=(ct == DP // 128 - 1),
                        tile_position=(0, 32 * (n % 4)))
            pa_sb = bv.tile([B, D], F32, tag="sB", name="pa_sb")
            for n in range(D // 512):
                nc.scalar.copy(pa_sb[:, n * 512:(n + 1) * 512],
                               o_ps[n // 4][32 * (n % 4):32 * (n % 4) + B, :])
            ar_in = dram.tile([B, D], F32)
            ar_out = dram.tile([B, D], F32)
            nc.scalar.dma_start(ar_in[:], pa_sb[:])
            nc.gpsimd.collective_compute(
                "AllReduce", OP.add,
                replica_groups=[list(range(NCORES))],
                ins=[ar_in.opt()], outs=[ar_out.opt()])

            # ---- phase 8: x_mid, rmsnorm2, transpose ----
            xpb = bv.tile([B, D], F32, tag="sA", name="xpb")
            nc.scalar.dma_start(xpb[:], xpb_g.ap())
            x_mid = bv.tile([B, D], F32, tag="sC", name="x_mid")
            nc.scalar.dma_start(x_mid[:], ar_out[:])
            nc.vector.tensor_add(x_mid[:], x_mid[:], xpb[:])
            nc.scalar.dma_start(xmid_out.ap(), x_mid[:])

            sq2 = bv.tile([B, D], F32, tag="sB", name="sq2")
            ssq2 = small.tile([B, 1], F32, tag="ssq2")
            nc.scalar.activation(sq2[:], x_mid[:], ACT_F.Square,
                                 accum_out=ssq2[:])
            rstd2 = small.tile([B, 1], F32, tag="rstd2")
            nc.scalar.activation(rstd2[:], ssq2[:], ACT_F.Sqrt,
                                 bias=eps_sb[:], scale=1.0 / D)
            nc.vector.reciprocal(rstd2[:], rstd2[:])
            xs2 = bv.tile([B, D], F32, tag="sA", name="xs2")
            nc.vector.tensor_scalar_mul(xs2[:], x_mid[:], rstd2[:])

            xn2T = actp.tile([128, KT, B], BF16, tag="xn2T")
            for kt in range(KT):
                tp = pt_pool.tile([128, B], F32, tag="tp", name="tp")
                nc.tensor.transpose(tp[:], xs2[:, kt * 128:(kt + 1) * 128],
                                    ident[:B, :B])
                nc.scalar.mul(xn2T[:, kt, :], tp[:], psw_sb[:, kt:kt + 1])

            # ---- phase 9a: h1 = xn2 @ w1, h3 = xn2 @ w3 ----
            h1_ps = pmm_pool.tile([128, 512], F32, tag="mm", name="h1_ps")
            h3_ps = pmm_pool.tile([128, 512], F32, tag="mm", name="h3_ps")
            for kt in range(KT):
                w1t = w1_pool.tile([128, FFP], BF16, tag="w1t", name="w1t")
                nc.sync.dma_start(w1t[:], w1.ap()[kt * 128:(kt + 1) * 128, :])
                w3t = w3_pool.tile([128, FFP], BF16, tag="w3t", name="w3t")
                nc.sync.dma_start(w3t[:], w3.ap()[kt * 128:(kt + 1) * 128, :])
                for n in range(FFP // 512):
                    nc.tensor.matmul(
                        h1_ps[32 * n:32 * n + B, :],
                        xn2T[:, kt, :], w1t[:, n * 512:(n + 1) * 512],
                        start=(kt == 0), stop=(kt == KT - 1),
                        tile_position=(0, 32 * n))
                    nc.tensor.matmul(
                        h3_ps[32 * n:32 * n + B, :],
                        xn2T[:, kt, :], w3t[:, n * 512:(n + 1) * 512],
                        start=(kt == 0), stop=(kt == KT - 1),
                        tile_position=(0, 32 * n))

            # ---- phase 9b: g = silu(h1) * h3, transpose to gT ----
            g_sb = firp.tile([B, FFP], F32, tag="firg", name="g_sb")
            s_sb = firp.tile([B, 512], F32, tag="zs", name="s_sb")
            for n in range(FFP // 512):
                nc.scalar.activation(s_sb[:], h1_ps[32 * n:32 * n + B, :],
                                     ACT_F.Silu)
                nc.vector.tensor_mul(g_sb[:, n * 512:(n + 1) * 512], s_sb[:],
                                     h3_ps[32 * n:32 * n + B, :])
            gT = actp.tile([128, KT2, B], BF16, tag="gT")
            for kt in range(KT2):
                tp = pt_pool.tile([128, B], F32, tag="tp", name="tp")
                nc.tensor.transpose(tp[:], g_sb[:, kt * 128:(kt + 1) * 128],
                                    ident[:B, :B])
                nc.scalar.copy(gT[:, kt, :], tp[:])

            # ---- phase 9c: mlp partial = g @ w2 ----
            m_ps = [pmm_pool.tile([128, 512], F32, tag="mm", name=f"m_ps{i}")
                    for i in range(2)]
            for kt in range(KT2):
                w2t = w2_pool.tile([128, D], BF16, tag="w2t", name="w2t")
                nc.sync.dma_start(w2t[:], w2.ap()[kt * 128:(kt + 1) * 128, :])
                for n in range(D // 512):
                    nc.tensor.matmul(
                        m_ps[n // 4][32 * (n % 4):32 * (n % 4) + B, :],
                        gT[:, kt, :], w2t[:, n * 512:(n + 1) * 512],
                        start=(kt == 0), stop=(kt == KT2 - 1),
                        tile_position=(0, 32 * (n % 4)))
            mlp_sb = bv.tile([B, D], F32, tag="sC", name="mlp_sb")
            for n in range(D // 512):
                nc.scalar.copy(mlp_sb[:, n * 512:(n + 1) * 512],
                               m_ps[n // 4][32 * (n % 4):32 * (n % 4) + B, :])
            nc.scalar.dma_start(mlp_out.ap(), mlp_sb[:])

    nc.compile()
    return nc


_NC_CACHE = None


def _get_program():
    global _NC_CACHE
    if _NC_CACHE is None:
        _NC_CACHE = build_program()
    return _NC_CACHE


def make_in_maps(x, fir_state, iir_state, pre_norm_w, proj_w, sf_weight,
                 sf_bias, D_res, residues, log_poles, out_w, out_b,
                 post_norm_w, mlp_w1, mlp_w3, mlp_w2):
    x2d = np.ascontiguousarray(np.asarray(x, np.float32)[:, -1, :])   # (B, D)
    xpb = x2d + np.asarray(out_b, np.float32)[None, :]
    poles = np.exp(np.asarray(log_poles, np.float64))[:, :, 0]        # (D, S)
    poles = poles.astype(np.float32)
    w = np.asarray(sf_weight, np.float32)[:, :FL]                     # (3D, FL)
    pnw_t = np.asarray(pre_norm_w, np.float32).reshape(D // 128, 128).T
    psw_t = np.asarray(post_norm_w, np.float32).reshape(D // 128, 128).T

    in_maps = []
    for k in range(NCORES):
        sl3 = slice(k * CP, (k + 1) * CP)
        sl1 = slice(k * DP, (k + 1) * DP)
        taps_k = np.concatenate([
            np.broadcast_to(w[sl3, 0], (B, CP)),
            np.broadcast_to(w[sl3, 1], (B, CP)),
            np.broadcast_to(w[sl3, 2], (B, CP)),
            np.broadcast_to(np.asarray(sf_bias, np.float32)[sl3], (B, CP)),
        ], axis=1)
        pol_k = np.broadcast_to(poles[sl1][:, None, :], (DP, B, S))
        res_k = np.broadcast_to(
            np.asarray(residues, np.float32)[sl1][:, None, :], (DP, B, S))
        in_maps.append({
            "x_g": x2d,
            "xpb_g": np.ascontiguousarray(xpb, dtype=np.float32),
            "projw": np.ascontiguousarray(proj_w[:, sl3]).astype(BF),
            "taps": np.ascontiguousarray(taps_k, dtype=np.float32),
            "fir_in": np.ascontiguousarray(
                np.asarray(fir_state, np.float32)[:, sl3, :]).reshape(B, 2 * CP),
            "polesT": np.ascontiguousarray(pol_k).reshape(DP, B * S),
            "residT": np.ascontiguousarray(res_k).reshape(DP, B * S),
            "dresT": np.ascontiguousarray(
                np.asarray(D_res, np.float32)[sl1]).reshape(DP, 1),
            "iir_in": np.ascontiguousarray(
                np.asarray(iir_state, np.float32)[:, sl1, :]),
            "outw": np.ascontiguousarray(out_w[sl1, :]).astype(BF),
            "pnw_t": np.ascontiguousarray(pnw_t),
            "psw_t": np.ascontiguousarray(psw_t),
            "w1": np.ascontiguousarray(mlp_w1[:, k * FFP:(k + 1) * FFP]).astype(BF),
            "w3": np.ascontiguousarray(mlp_w3[:, k * FFP:(k + 1) * FFP]).astype(BF),
            "w2": np.ascontiguousarray(mlp_w2[k * FFP:(k + 1) * FFP, :]).astype(BF),
        })
    return in_maps


def kernel(**inputs):
    nc = _get_program()
    in_maps = make_in_maps(**inputs)
    res = run_bass_kernel_spmd(nc, in_maps, list(range(NCORES)))
    r = res.results

    out_x = r[0]["xmid_out"].astype(np.float64)
    for c in range(NCORES):
        out_x += r[c]["mlp_out"].astype(np.float64)
    out_x = out_x.astype(np.float32)[:, None, :]                  # (B, 1, D)

    fir_state = np.asarray(inputs["fir_state"], np.float32)
    new_fir = np.empty((B, 3 * D, FL - 1), np.float32)
    new_fir[:, :, 0] = fir_state[:, :, 1]
    new_fir[:, :, 1] = np.concatenate(
        [r[c]["u_out"] for c in range(NCORES)], axis=1)
    new_iir = np.concatenate(
        [r[c]["iir_out"] for c in range(NCORES)], axis=1)        # (B, D, S)
    return out_x, new_fir, new_iir
